# revision 16
# baseline (speedup 1.0000x reference)
"""
Trainium2 Bass kernel for nn_Encoder (embedding lookup + LSTM, returns final (h, c)).

Strategy (data-parallel over batch, per sharding hint):
  - 8 cores, each handles B_local = 4 of the 32 batch rows.
  - Per core: gather embedding rows via indirect DMA (t-major order),
    transpose on PE, project x @ W with fp32r matmuls (chunked over T),
    then run the 512-step recurrence with U as the stationary operand
    in fp16 (FWL 2x weight loads) producing gates transposed
    (4H on partitions) so activations/cell update run wide on ACT/DVE.
  - Gate layout: psum z tile per H-slice hs (4 of them, one PSUM bank each),
    packed columns (gate', b) with gate' order (i, f, o, g) so one sigmoid
    covers i,f,o and one tanh covers g.
  - h is kept as hT [128 x (hs, b)] fp16 which is exactly the moving-operand
    layout the next step's matmuls need.

Host side: shard/marshal inputs, run SPMD on 8 cores, unpack outputs.
"""

import numpy as np

import concourse.bass as bass
import concourse.mybir as mybir
import concourse.tile as tile
from concourse import bacc
from concourse.bass import IndirectOffsetOnAxis
from concourse.bass_utils import run_bass_kernel_spmd
from concourse.masks import make_identity

# Problem constants (hardcoded; harness contract)
B, T, V, E, H = 32, 512, 20000, 300, 512
G4 = 4 * H            # 2048
NCORES = 8
BL = B // NCORES      # 4 batch rows per core
P = 128
KM = G4 // P          # 16 M-tiles over 4H
KH = H // P           # 4 K-tiles over H
KE_SIZES = [128, 128, 44]   # K subtiles over E=300
# Keras gate g (i,f,g,o) -> packed slot (i,f,o,g): sigmoid = slots 0..2, tanh = slot 3
PERM = [0, 1, 3, 2]

f32 = mybir.dt.float32
f32r = mybir.dt.float32r
f16 = mybir.dt.float16
i32 = mybir.dt.int32

AF = mybir.ActivationFunctionType


def build_program(nc, T_steps=T, Tc=128, dbg_step=None, reps=1):
    """Emit the full per-core program into nc (a bacc.Bacc).

    reps > 1 repeats the whole compute (for timing amplification)."""
    assert T_steps % Tc == 0
    nch = T_steps // Tc
    NJ = Tc * BL // P  # gathers (128-row groups) per chunk

    emb_t = nc.declare_dram_parameter("emb", [V, E], f32, isOutput=False)
    W_t = nc.declare_dram_parameter("W", [E, G4], f32, isOutput=False)
    U_t = nc.declare_dram_parameter("U", [H, G4], f32, isOutput=False)
    b_t = nc.declare_dram_parameter("bvec", [G4], f32, isOutput=False)
    tok_t = nc.declare_dram_parameter("tok", [P, T_steps * BL // P], i32, isOutput=False)
    ho_t = nc.declare_dram_parameter("ho", [P, BL * KH], f16, isOutput=True)
    co_t = nc.declare_dram_parameter("co", [P, BL * KH], f32, isOutput=True)
    if dbg_step is not None:
        dbg_z = nc.declare_dram_parameter("dbg_z", [P, 64], f32, isOutput=True)
        dbg_h = nc.declare_dram_parameter("dbg_h", [P, BL * KH], f16, isOutput=True)
        dbg_c = nc.declare_dram_parameter("dbg_c", [P, BL * KH], f32, isOutput=True)

    with tile.TileContext(nc) as tc:
        with (
            tc.tile_pool(name="const", bufs=1) as cpool,
            tc.tile_pool(name="ustage", bufs=2) as upool,
            tc.tile_pool(name="xrows", bufs=4) as xpool,
            tc.tile_pool(name="xtp", bufs=2) as xtpool,
            tc.tile_pool(name="ptr", bufs=2, space="PSUM") as ptr_pool,
            tc.tile_pool(name="pxz", bufs=2, space="PSUM") as pxz_pool,
            tc.tile_pool(name="pz", bufs=4, space="PSUM") as pz_pool,
        ):
            # ---- constants / weights ----
            U16 = cpool.tile([P, KH * G4], f16, tag="U16")
            W_sb = cpool.tile([P, 3 * G4], f16, tag="Wsb")
            b_sb = cpool.tile([P, KM], f32, tag="bsb")
            tok_sb = cpool.tile([P, T_steps * BL // P], i32, tag="tok")
            ident = cpool.tile([P, P], f32, tag="ident")
            h16 = cpool.tile([P, BL * KH], f16, tag="h16")
            cst = cpool.tile([P, BL * KH], f32, tag="cst")
            z_s = cpool.tile([P, 64], f32, tag="zs")
            a_s = cpool.tile([P, 64], f32, tag="as")
            tmp1 = cpool.tile([P, BL * KH], f32, tag="t1")
            tmp2 = cpool.tile([P, BL * KH], f32, tag="t2")
            tct = cpool.tile([P, BL * KH], f32, tag="tct")
            xz_sb = [
                cpool.tile([P, Tc * 64], f32, tag=f"xz{par}", name=f"xz{par}")
                for par in range(2)
            ]

            make_identity(nc, ident[:])

            # U (fp32 DRAM) -> U16 (fp16 SBUF), K-tile k region at cols k*G4
            for k in range(KH):
                ust = upool.tile([P, G4], f32, tag="ustage")
                nc.sync.dma_start(ust[:], U_t.ap()[k * P:(k + 1) * P, :])
                nc.vector.tensor_copy(U16[:, k * G4:(k + 1) * G4], ust[:])

            # W: 3 K-subtiles at cols kk*G4, cast to fp16 via staging
            ofs = 0
            for kk, kw in enumerate(KE_SIZES):
                wst = upool.tile([P, G4], f32, tag="ustage", name=f"wst{kk}")
                nc.sync.dma_start(wst[:kw, :], W_t.ap()[ofs:ofs + kw, :])
                nc.vector.tensor_copy(W_sb[:kw, kk * G4:(kk + 1) * G4], wst[:kw, :])
                ofs += kw

            # bias: b_sb[p, m] = b[m*128 + p]
            nc.sync.dma_start(b_sb[:], b_t.ap().rearrange("(m p) -> p m", p=P))
            nc.sync.dma_start(tok_sb[:], tok_t.ap())

            nc.gpsimd.memset(h16[:], 0.0)
            nc.gpsimd.memset(cst[:], 0.0)

            def emit_prep(c):
                """Gather + transpose + xz projection for chunk c."""
                xz_dst = xz_sb[c % 2]
                xT = xtpool.tile([P, 3 * Tc * BL], f16, tag="xT")
                for j in range(NJ):
                    xr = xpool.tile([P, E], f32, tag="xrows")
                    nc.gpsimd.indirect_dma_start(
                        out=xr[:],
                        out_offset=None,
                        in_=emb_t.ap(),
                        in_offset=IndirectOffsetOnAxis(
                            ap=tok_sb[:, c * NJ + j:c * NJ + j + 1], axis=0
                        ),
                    )
                    for kk, kw in enumerate(KE_SIZES):
                        pt = ptr_pool.tile([P, P], f32, tag="ptr")
                        nc.tensor.transpose(
                            out=pt[:kw, :], in_=xr[:, kk * P:kk * P + kw],
                            identity=ident[:],
                        )
                        nc.vector.tensor_copy(
                            xT[:kw, kk * Tc * BL + j * P:kk * Tc * BL + (j + 1) * P],
                            pt[:kw, :],
                        )
                N = Tc * BL
                for m in range(KM):
                    pxz = pxz_pool.tile([P, N], f32, tag="pxz")
                    for kk, kw in enumerate(KE_SIZES):
                        nc.tensor.matmul(
                            pxz[:],
                            W_sb[:kw, kk * G4 + m * P:kk * G4 + (m + 1) * P],
                            xT[:kw, kk * N:(kk + 1) * N],
                            start=(kk == 0),
                            stop=(kk == 2),
                        )
                    # packed dest: col = t*64 + (m%4)*16 + PERM[m//4]*4 + b
                    slot = (m % 4) * 16 + PERM[m // 4] * 4
                    dst = xz_dst[:].rearrange("p (t g) -> p t g", g=64)[
                        :, :, slot:slot + 4
                    ]
                    src = pxz[:].rearrange("p (t b) -> p t b", b=BL)
                    nc.vector.tensor_scalar_add(dst, src, b_sb[:, m:m + 1])

            # MM emission order for the last K round: group M-tiles by H-slice
            ORDER_LAST = [m for hs in range(4) for m in (hs, 4 + hs, 8 + hs, 12 + hs)]

            def emit_step(c, t):
                psz = [
                    pz_pool.tile([P, 16], f32, tag="pz", name=f"pz{hs}_{c}_{t}")
                    for hs in range(4)
                ]
                for k in range(KH):
                    order = ORDER_LAST if k == KH - 1 else range(KM)
                    for m in order:
                        slot = PERM[m // 4] * 4
                        # start=True marks the whole 2KB psum bank pending-zero,
                        # so only the FIRST matmul touching each psz tile sets it
                        # (round k=0, m in 0..3); later slots overwrite via
                        # pending-zero, later k rounds accumulate.
                        nc.tensor.matmul(
                            psz[m % 4][:, slot:slot + 4],
                            U16[:, k * G4 + m * P:k * G4 + (m + 1) * P],
                            h16[:, k * BL:(k + 1) * BL],
                            start=(k == 0 and m < 4),
                            stop=(k == KH - 1),
                            skip_group_check=True,
                        )
                for hs in range(4):
                    zs = z_s[:, hs * 16:hs * 16 + 16]
                    nc.vector.tensor_add(
                        zs,
                        psz[hs][:],
                        xz_sb[c % 2][:, t * 64 + hs * 16:t * 64 + hs * 16 + 16],
                    )
                    # sigmoid over (i, f, o) slots, tanh over g slot
                    nc.scalar.activation(
                        a_s[:, hs * 16:hs * 16 + 12], z_s[:, hs * 16:hs * 16 + 12],
                        AF.Sigmoid,
                    )
                    nc.scalar.activation(
                        a_s[:, hs * 16 + 12:hs * 16 + 16],
                        z_s[:, hs * 16 + 12:hs * 16 + 16],
                        AF.Tanh,
                    )
                    cs = slice(hs * BL, (hs + 1) * BL)
                    nc.vector.tensor_mul(
                        tmp1[:, cs], a_s[:, hs * 16 + 4:hs * 16 + 8], cst[:, cs]
                    )  # f * c
                    nc.vector.tensor_mul(
                        tmp2[:, cs],
                        a_s[:, hs * 16:hs * 16 + 4],
                        a_s[:, hs * 16 + 12:hs * 16 + 16],
                    )  # i * g
                    nc.vector.tensor_add(cst[:, cs], tmp1[:, cs], tmp2[:, cs])
                    nc.scalar.activation(tct[:, cs], cst[:, cs], AF.Tanh)
                    nc.vector.tensor_mul(
                        h16[:, cs], a_s[:, hs * 16 + 8:hs * 16 + 12], tct[:, cs]
                    )  # h = o * tanh(c), cast to fp16 on write

            for rep in range(reps):
                if rep > 0:
                    nc.gpsimd.memset(h16[:], 0.0)
                    nc.gpsimd.memset(cst[:], 0.0)
                emit_prep(0)
                for c in range(nch):
                    for t in range(Tc):
                        emit_step(c, t)
                        if dbg_step is not None and (c, t) == dbg_step:
                            nc.sync.dma_start(dbg_z.ap(), z_s[:])
                            nc.sync.dma_start(dbg_h.ap(), h16[:])
                            nc.sync.dma_start(dbg_c.ap(), cst[:])
                        if t == 16 and c + 1 < nch:
                            emit_prep(c + 1)

            nc.sync.dma_start(ho_t.ap(), h16[:])
            nc.sync.dma_start(co_t.ap(), cst[:])

    return nc


_CACHE = {}


def _get_compiled(T_steps=T, Tc=128):
    key = (T_steps, Tc)
    if key not in _CACHE:
        nc = bacc.Bacc(None, target_bir_lowering=False)
        build_program(nc, T_steps, Tc)
        nc.compile()
        _CACHE[key] = nc
    return _CACHE[key]


def make_tok_idx(tokens_slice, T_steps=T):
    """tokens_slice [BL, T] -> [128, T*BL/128] int32, [p, j] = t-major flat[j*128+p]."""
    flat = tokens_slice.T.reshape(-1)  # index n = t*BL + b
    return np.ascontiguousarray(
        flat.reshape(T_steps * BL // P, P).T.astype(np.int32)
    )


def unpack_state(arr):
    """[128, 16] packed (p, hs*4+b) -> [BL, H]."""
    a = np.asarray(arr).astype(np.float32).reshape(P, KH, BL)
    return a.transpose(2, 1, 0).reshape(BL, H)


def kernel(tokens, emb, W, U, b):
    tokens = np.ascontiguousarray(np.asarray(tokens), dtype=np.int32)
    emb = np.ascontiguousarray(np.asarray(emb), dtype=np.float32)
    W = np.ascontiguousarray(np.asarray(W), dtype=np.float32)
    U = np.ascontiguousarray(np.asarray(U), dtype=np.float32)
    b = np.ascontiguousarray(np.asarray(b), dtype=np.float32)

    nc = _get_compiled()
    in_maps = []
    for i in range(NCORES):
        in_maps.append(
            {
                "emb": emb,
                "W": W,
                "U": U,
                "bvec": b,
                "tok": make_tok_idx(tokens[i * BL:(i + 1) * BL]),
            }
        )
    res = run_bass_kernel_spmd(nc, in_maps, core_ids=list(range(NCORES))).results

    h = np.zeros((B, H), np.float32)
    c = np.zeros((B, H), np.float32)
    for i in range(NCORES):
        h[i * BL:(i + 1) * BL] = unpack_state(res[i]["ho"])
        c[i * BL:(i + 1) * BL] = unpack_state(res[i]["co"])
    return h, c


def _build_run_fn(nc):
    """jit'd fn running the kernel once on 8 cores (device-resident args)."""
    import jax
    from jax.sharding import Mesh, PartitionSpec
    from jax.experimental.shard_map import shard_map
    import concourse.mybir as mybir_
    from concourse import bass2jax

    bass2jax.install_neuronx_cc_hook()

    partition_name = nc.partition_id_tensor.name if nc.partition_id_tensor else None
    in_names, out_names, out_avals = [], [], []
    for alloc in nc.m.functions[0].allocations:
        if not isinstance(alloc, mybir_.MemoryLocationSet):
            continue
        name = alloc.memorylocations[0].name
        if alloc.kind == "ExternalInput":
            if name != partition_name:
                in_names.append(name)
        elif alloc.kind == "ExternalOutput":
            out_names.append(name)
            out_avals.append(
                jax.core.ShapedArray(
                    tuple(alloc.tensor_shape), mybir_.dt.np(alloc.dtype)
                )
            )
    n_params = len(in_names)
    all_in_names = list(in_names) + list(out_names)
    if partition_name is not None:
        all_in_names.append(partition_name)

    def _body(*args):
        operands = list(args)
        if partition_name is not None:
            operands.append(bass2jax.partition_id_tensor())
        return tuple(
            bass2jax._bass_exec_p.bind(
                *operands,
                out_avals=tuple(out_avals),
                in_names=tuple(all_in_names),
                out_names=tuple(out_names),
                lowering_input_output_aliases=(),
                sim_require_finite=True,
                sim_require_nnan=True,
                nc=nc,
            )
        )

    devices = jax.devices()[:NCORES]
    mesh = Mesh(np.asarray(devices), ("core",))
    nio = n_params + len(out_names)
    fn = jax.jit(
        shard_map(
            _body,
            mesh=mesh,
            in_specs=(PartitionSpec("core"),) * nio,
            out_specs=(PartitionSpec("core"),) * len(out_names),
            check_rep=False,
        )
    )
    return fn, in_names, out_names, out_avals


def _min_wall(nc, in_maps, calls=12):
    """Min wall-clock of a device-resident execution of nc across `calls`."""
    import time as _time
    import jax

    fn, in_names, out_names, out_avals = _build_run_fn(nc)
    concat_in = [
        np.concatenate([in_maps[c][k] for c in range(NCORES)], axis=0)
        for k in in_names
    ]
    concat_zeros = [
        np.zeros((NCORES * a.shape[0], *a.shape[1:]), a.dtype) for a in out_avals
    ]
    args = [jax.device_put(x) for x in concat_in + concat_zeros]
    o = fn(*args)
    jax.block_until_ready(o)  # compile + first exec (instruction stream cold)
    walls = []
    for _ in range(calls):
        t0 = _time.perf_counter()
        o = fn(*args)
        jax.block_until_ready(o)
        walls.append(_time.perf_counter() - t0)
    walls.sort()
    return walls[0], walls[len(walls) // 2]


def _make_in_maps(np_inputs):
    tokens = np.ascontiguousarray(np.asarray(np_inputs["tokens"]), dtype=np.int32)
    in_maps = []
    for i in range(NCORES):
        in_maps.append(
            {
                "emb": np.asarray(np_inputs["emb"], np.float32),
                "W": np.asarray(np_inputs["W"], np.float32),
                "U": np.asarray(np_inputs["U"], np.float32),
                "bvec": np.asarray(np_inputs["b"], np.float32),
                "tok": make_tok_idx(tokens[i * BL:(i + 1) * BL]),
            }
        )
    return in_maps


def time_kernel_hw(np_inputs, reps_hi=3, calls=12):
    """Estimate one-pass HW time (ns): difference of amplified variants.

    Builds the kernel with the whole compute repeated 1x and reps_hi x,
    times device-resident executions of both, and divides the delta —
    dispatch/transfer overhead cancels.
    """
    in_maps = _make_in_maps(np_inputs)

    variants = {}
    for r in (1, reps_hi):
        nc = bacc.Bacc(None, target_bir_lowering=False)
        build_program(nc, T, 128, reps=r)
        nc.compile()
        variants[r] = nc

    lo_min, lo_med = _min_wall(variants[1], in_maps, calls)
    hi_min, hi_med = _min_wall(variants[reps_hi], in_maps, calls)
    per_pass_min = (hi_min - lo_min) / (reps_hi - 1)
    per_pass_med = (hi_med - lo_med) / (reps_hi - 1)
    print(
        f"timing: R1 min/med {lo_min * 1e3:.2f}/{lo_med * 1e3:.2f} ms, "
        f"R{reps_hi} min/med {hi_min * 1e3:.2f}/{hi_med * 1e3:.2f} ms, "
        f"per-pass min/med {per_pass_min * 1e3:.3f}/{per_pass_med * 1e3:.3f} ms"
    )
    return max(per_pass_min, 0.0) * 1e9

